# revision 27
# baseline (speedup 1.0000x reference)
"""Causal attention layer (B=4, N=2048, C=1024, H=16, D=64) on 8 TRN2 NeuronCores.

Sharding: core c -> (batch b = c//2, head-group g = c%2 of 8 heads).

All PE operands are bf16 (halves LDWEIGHTS time vs fp32r; no fp32r narrow-
moving penalty). Every matmul dst is confined to one PSUM bank (512 f32 cols).

  qkv   : per o-tile / 512-col half: accumulate 8 [128,128] w-chunks against
          x chunks -> psum [128,512] -> sbuf (bf16).
  attn  : per (head, 1024-q-megablock): k-outer loop. S_k = kT-tile^T qT in
          1-2 bank-piece matmuls (ap shrinks toward the diagonal), ONE
          full-width exp per k on ScalarE, tril-mask on DVE for diagonal
          tiles, AV accumulates [1|v]^T P into one psum [128,1024] with
          per-piece suffix ranges (causality at 128-key granularity). Each
          512-half is normalized as soon as its last AV lands so proj can
          start early.
  proj  : flipped: stationary = proj chunk, moving = attn_outT -> output is
          TRANSPOSED [C, N]; host transposes back (host time is free).

qkv/transpose/proj work is queued as single-matmul filler closures and
interleaved into the attention k-loop so the PE never waits on ScalarE.
"""
import sys

sys.path.insert(0, "/opt/trn_rl_repo")

import numpy as np

import concourse.bass as bass  # noqa: F401
import concourse.tile as tile
from concourse import bacc, mybir
from concourse.bass_utils import run_bass_kernel_spmd

F32 = mybir.dt.float32
BF16 = mybir.dt.bfloat16
EXP = mybir.ActivationFunctionType.Exp

B, N, C, H, D = 4, 2048, 1024, 16, 64
G = 8            # heads per core
GC = G * D       # 512 channels per core
NT = N // 128    # 16 k-tiles
CK = C // 128    # 8 contraction chunks
MB = 1024        # q-megablock width
SCALE = float(D) ** -0.5

_cache = {}


def _bank_pieces(c0, c1):
    pieces = []
    c = c0
    while c < c1:
        e = min(c1, (c // 512 + 1) * 512)
        pieces.append((c, e))
        c = e
    return pieces


def _build_nc():
    from contextlib import ExitStack

    nc = bacc.Bacc("TRN2", target_bir_lowering=False, debug=False)

    xT_d = nc.dram_tensor("xT", [C, N], BF16, kind="ExternalInput")
    # host pre-tiles qkv weights to [p, ot, cc, oo] so the DMA is contiguous
    wqkvT_d = nc.dram_tensor("wqkvT", [128, 12 * CK * 128], BF16,
                             kind="ExternalInput")
    projT_d = nc.dram_tensor("projT", [GC, C], BF16, kind="ExternalInput")
    tril_d = nc.dram_tensor("tril", [128, 128], BF16, kind="ExternalInput")
    ident_d = nc.dram_tensor("ident", [128, 128], BF16, kind="ExternalInput")
    onesb_d = nc.dram_tensor("onesb", [128, NT], BF16, kind="ExternalInput")
    outT_d = nc.dram_tensor("outT", [C, N], F32, kind="ExternalOutput")

    with tile.TileContext(nc) as tc:
        with ExitStack() as ctx:
            consts = ctx.enter_context(tc.tile_pool(name="consts", bufs=1))
            wt_pool = ctx.enter_context(tc.tile_pool(name="wt", bufs=1))
            xs_pool = ctx.enter_context(tc.tile_pool(name="xs", bufs=1))
            qk_pool = ctx.enter_context(tc.tile_pool(name="qk", bufs=6))
            vT_pool = ctx.enter_context(tc.tile_pool(name="vT", bufs=2))
            vext_pool = ctx.enter_context(tc.tile_pool(name="vext", bufs=1))
            pt_pool = ctx.enter_context(tc.tile_pool(name="pt", bufs=4))
            aoT_pool = ctx.enter_context(tc.tile_pool(name="aoT", bufs=1))
            pj_pool = ctx.enter_context(tc.tile_pool(name="pj", bufs=1))
            os_pool = ctx.enter_context(tc.tile_pool(name="os", bufs=3))
            rf_pool = ctx.enter_context(tc.tile_pool(name="rf", bufs=3))
            bcs_pool = ctx.enter_context(tc.tile_pool(name="bcs", bufs=3))
            tmp_pool = ctx.enter_context(tc.tile_pool(name="tmp", bufs=3))
            ob_pool = ctx.enter_context(tc.tile_pool(name="ob", bufs=3))
            psS = ctx.enter_context(tc.tile_pool(name="psS", bufs=2, space="PSUM"))
            psO = ctx.enter_context(tc.tile_pool(name="psO", bufs=1, space="PSUM"))
            psF = ctx.enter_context(tc.tile_pool(name="psF", bufs=2, space="PSUM"))

            # Input DMA descriptor generation costs ~600ns per dma_start on
            # an engine sequencer. Spread input loads across queues that are
            # idle at the prologue, giving each engine only what must land
            # before that engine's first compute op.
            tril_sb = consts.tile([128, 128], BF16)
            ident_sb = consts.tile([128, 128], BF16)
            wt_all = wt_pool.tile([128, 12, CK * 128], BF16, tag="wt",
                                  name="wt")
            xs = xs_pool.tile([128, CK, N], BF16, tag="xs", name="xs")
            v_ext = [vext_pool.tile([128, NT, 128], BF16, tag=f"ve{h}",
                                    name=f"ve{h}")
                     for h in range(G)]
            pj_sb = pj_pool.tile([128, 4, C], BF16, tag="pj", name="pj")

            def load_w(q, ot):
                q.dma_start(wt_all[:, ot, :],
                            wqkvT_d[:, 1024 * ot:1024 * (ot + 1)])

            def w_stat(ot, cc):
                return wt_all[:, ot, 128 * cc:128 * (cc + 1)]

            def load_x_all(q, c0):
                # one dma_start per 512-col block of all 8 cc chunks:
                # descriptor generation is the bottleneck, not bandwidth
                q.dma_start(
                    xs[:, :, c0:c0 + 512],
                    xT_d[:, c0:c0 + 512].rearrange("(cc p) n -> p cc n",
                                                   p=128),
                )

            # vector queue: consts + v0 weights + late v_ext ones (free
            # until the first qkv-psum copy at ~12us)
            # scalar queue: prologue-critical consts/weights + pair0 ones
            # (only engines SP/Activation/gpsimd may initiate DMAs)
            nc.scalar.dma_start(tril_sb[:], tril_d[:])
            nc.scalar.dma_start(ident_sb[:], ident_d[:])
            load_w(nc.scalar, 8)
            load_w(nc.scalar, 0)
            load_w(nc.scalar, 4)
            for h in range(2):
                nc.scalar.dma_start(v_ext[h][:, :, 0:1], onesb_d[:, :, None])
            # gpsimd queue: x megablock 0 (free until first broadcast ~20us)
            load_x_all(nc.gpsimd, 0)
            load_x_all(nc.gpsimd, 512)
            # sync queue: the rest (x mb1 first: fillers need it ~20us)
            load_x_all(nc.sync, 1024)
            load_x_all(nc.sync, 1536)
            for h in range(2, G):
                nc.sync.dma_start(v_ext[h][:, :, 0:1], onesb_d[:, :, None])
            for ot in (9, 1, 5, 10, 2, 6, 11, 3, 7):
                load_w(nc.sync, ot)
            for gcc in range(4):
                nc.sync.dma_start(
                    pj_sb[:, gcc, :], projT_d[128 * gcc:128 * (gcc + 1), :]
                )

            aoT = [aoT_pool.tile([128, N], BF16, tag=f"ao{p}", name=f"ao{p}")
                   for p in range(4)]

            # ---------------- filler units (1 PE op per closure) -----------
            def qkv_unit(ot, c0, dst):
                """8 closures: accumulate psum [128,512], copy to dst cols."""
                cell = {}
                steps = []
                for cc in range(CK):
                    def _mm(cc=cc, ot=ot, c0=c0, dst=dst):
                        if cc == 0:
                            cell["ps"] = psF.tile([128, 512], F32, tag="F",
                                                  name=f"ps{ot}_{c0}")
                        nc.tensor.matmul(
                            cell["ps"][:],
                            w_stat(ot, cc),
                            xs[:, cc, c0:c0 + 512],
                            start=(cc == 0),
                            stop=(cc == CK - 1),
                        )
                        if cc == CK - 1:
                            nc.vector.tensor_copy(
                                dst[:, c0:c0 + 512], cell["ps"][:]
                            )
                    steps.append(_mm)
                return steps

            def tr_unit(vp, vt, nt):
                """1 closure: transpose one 128-col v tile into v_ext."""
                def _tr(vp=vp, vt=vt, nt=nt):
                    tp = psF.tile([128, 128], BF16, tag="F", name="tp")
                    nc.tensor.transpose(
                        tp[:], vt[:, 128 * nt:128 * (nt + 1)], ident_sb[:]
                    )
                    nc.vector.tensor_copy(
                        v_ext[2 * vp][:, nt, 64:128], tp[:, 0:64]
                    )
                    nc.vector.tensor_copy(
                        v_ext[2 * vp + 1][:, nt, 64:128], tp[:, 64:128]
                    )
                return [_tr]

            def proj_unit(co, c0, pool=None, tag="F"):
                """4 closures: accumulate 4 gc-chunks, copy+DMA out transposed."""
                cell = {}
                steps = []
                for gcc in range(4):
                    def _mm(gcc=gcc, co=co, c0=c0, pool=pool, tag=tag):
                        if gcc == 0:
                            cell["ps"] = (pool or psF).tile(
                                [128, 512], F32, tag=tag, name=f"pp{co}_{c0}"
                            )
                        nc.tensor.matmul(
                            cell["ps"][:],
                            pj_sb[:, gcc, 128 * co:128 * (co + 1)],
                            aoT[gcc][:, c0:c0 + 512],
                            start=(gcc == 0),
                            stop=(gcc == 3),
                        )
                        if gcc == 3:
                            ob = ob_pool.tile([128, 512], F32, tag="ob",
                                              name="ob")
                            nc.vector.tensor_copy(ob[:], cell["ps"][:])
                            # mb1 units run at the tail when ScalarE is idle:
                            # alternate queues there so the final drain isn't
                            # serialized on one descriptor generator
                            dq = nc.scalar if c0 >= MB and co % 2 else nc.sync
                            dq.dma_start(
                                outT_d[128 * co:128 * (co + 1), c0:c0 + 512],
                                ob[:],
                            )
                    steps.append(_mm)
                return steps

            pending = []

            def fill(n):
                for _ in range(min(n, len(pending))):
                    pending.pop(0)()

            # ---------------- attention chain ------------------------------
            def chain(p, h, mb, qT, kT, fps):
                hg = 2 * p + h
                hh = slice(64 * h, 64 * (h + 1))
                kmax = 8 * (mb + 1)
                klo = 8 * mb + 3      # last k writing cols [0,512)
                Ps = [None] * kmax
                c0s = [0] * kmax

                def emit_s_exp(k):
                    c0 = max(0, 128 * k - MB * mb)
                    c0s[k] = c0
                    w = MB - c0
                    S = psS.tile([128, MB], F32, tag="S", name=f"S{k}")
                    for a, b in _bank_pieces(c0, MB):
                        nc.tensor.matmul(
                            S[:, a:b],
                            kT[hh, 128 * k:128 * (k + 1)],
                            qT[hh, MB * mb + a:MB * mb + b],
                        )
                    P = pt_pool.tile([128, MB], BF16, tag="P", name=f"P{k}")
                    nc.scalar.activation(P[:, c0:MB], S[:, c0:MB], EXP,
                                         scale=SCALE)
                    if 128 * k >= MB * mb:
                        # on GpSimd: keeps the exp->mask->AV chain off the
                        # DVE queue, which is busy with bulk psum copies
                        nc.gpsimd.tensor_mul(P[:, c0:c0 + 128],
                                             P[:, c0:c0 + 128], tril_sb[:])
                    Ps[k] = P

                def norm_half(oT, hf):
                    oS = os_pool.tile([128, 512], F32, tag="os", name="oS")
                    nc.vector.tensor_copy(oS[:], oT[:, 512 * hf:512 * (hf + 1)])
                    Rf = rf_pool.tile([1, 512], F32, tag="rf", name="Rf")
                    nc.vector.reciprocal_approx_fast(Rf[:], oS[0:1, :])
                    bcs = bcs_pool.tile([128, 512], F32, tag="bcs", name="bcs")
                    nc.gpsimd.partition_broadcast(bcs[:], Rf[:])
                    tmp = tmp_pool.tile([128, 512], BF16, tag="tmp", name="tmp")
                    nc.vector.tensor_mul(tmp[64:128, :], oS[64:128, :],
                                         bcs[64:128, :])
                    nc.sync.dma_start(
                        aoT[p][64 * h:64 * (h + 1),
                               MB * mb + 512 * hf:MB * mb + 512 * (hf + 1)],
                        tmp[64:128, :],
                    )

                oT = psO.tile([128, MB], F32, tag="O", name="oT")
                emit_s_exp(0)
                if kmax > 1:
                    emit_s_exp(1)
                for k in range(kmax):
                    fill(fps)
                    c0 = c0s[k]
                    for a, b in _bank_pieces(c0, MB):
                        stop = (k == klo and a < 512) or \
                               (k == kmax - 1 and b > 512)
                        nc.tensor.matmul(
                            oT[:, a:b],
                            v_ext[hg][:, k, :],
                            Ps[k][:, a:b],
                            start=(k == 0),
                            stop=stop,
                            skip_group_check=True,
                        )
                    if k + 2 < kmax:
                        emit_s_exp(k + 2)
                    if k == klo:
                        norm_half(oT, 0)
                if kmax == 1:
                    norm_half(oT, 0)
                norm_half(oT, 1)

            # ---------------- prologue -------------------------------------
            qTs, kTs, vTs = {}, {}, {}
            for p in range(4):
                qTs[p] = qk_pool.tile([128, N], BF16, tag="qk", name=f"q{p}")
                kTs[p] = qk_pool.tile([128, N], BF16, tag="qk", name=f"k{p}")
            for p in range(4):
                vTs[p] = vT_pool.tile([128, N], BF16, tag="vt", name=f"v{p}")

            def emit(steps):
                for s in steps:
                    s()

            emit(qkv_unit(8, 0, vTs[0]))               # v0 q-half 0
            emit(qkv_unit(8, 512, vTs[0]))             # v0 q-half 1
            emit(qkv_unit(0, 0, qTs[0]))               # q0 half 0
            emit(qkv_unit(0, 512, qTs[0]))             # q0 half 1
            emit(qkv_unit(4, 0, kTs[0]))               # k0 half 0
            emit(qkv_unit(4, 512, kTs[0]))             # k0 half 1
            for nt in range(4):
                emit(tr_unit(0, vTs[0], nt))

            # remainder of pair0's deps (60 steps), consumed early in pair0
            for nt in range(4, 8):
                pending += tr_unit(0, vTs[0], nt)
            for c0 in (1024, 1536):
                pending += qkv_unit(8, c0, vTs[0])
            for c0 in (1024, 1536):
                pending += qkv_unit(0, c0, qTs[0])
            for c0 in (1024, 1536):
                pending += qkv_unit(4, c0, kTs[0])
            for nt in range(8, 16):
                pending += tr_unit(0, vTs[0], nt)
            # pair p block (112 steps each) drains during pair p-1
            for p in range(1, 4):
                for c0 in (0, 512, 1024, 1536):
                    pending += qkv_unit(8 + p, c0, vTs[p])
                for c0 in (0, 512, 1024, 1536):
                    pending += qkv_unit(p, c0, qTs[p])
                for nt in range(NT):
                    pending += tr_unit(p, vTs[p], nt)
                for c0 in (0, 512, 1024, 1536):
                    pending += qkv_unit(4 + p, c0, kTs[p])

            # ---------------- pair loop ------------------------------------
            # fills per attention k-step; pair0 drains 172 queued steps
            # (its 60-step tail + pair1's 112), pairs 1-2 the next 112-block,
            # pair3 the proj-mb0 block (64) during its mb1 chains.
            FPS = {0: (4, 4), 1: (3, 2), 2: (3, 2), 3: (0, 2)}
            for p in range(4):
                for mb in (0, 1):
                    for h in (0, 1):
                        chain(p, h, mb, qTs[p], kTs[p], FPS[p][mb])
                    if p == 3:
                        # mb1 units run at the tail: rotate through the
                        # then-idle psS bufs too, so units double-buffer.
                        for i, (co, hf) in enumerate(
                            (co, hf) for co in range(CK) for hf in (0, 1)
                        ):
                            if mb == 0 or i % 2 == 0:
                                pending += proj_unit(co, MB * mb + 512 * hf)
                            else:
                                pending += proj_unit(co, MB * mb + 512 * hf,
                                                     pool=psS, tag="S")
            fill(len(pending))

    nc.compile()
    return nc


def _tril_np():
    import ml_dtypes

    i = np.arange(128)[:, None]
    j = np.arange(128)[None, :]
    return (j >= i).astype(np.float32).astype(ml_dtypes.bfloat16)


def make_in_maps(x, qkv_w, proj_w):
    import ml_dtypes

    bf16 = ml_dtypes.bfloat16
    x = np.asarray(x, dtype=np.float32)
    qkv_w = np.asarray(qkv_w, dtype=np.float32)
    proj_w = np.asarray(proj_w, dtype=np.float32)
    tril = _tril_np()
    ident = np.eye(128, dtype=np.float32).astype(bf16)
    onesb = np.ones((128, NT), dtype=np.float32).astype(bf16)
    in_maps = []
    for c in range(8):
        b, g = c // 2, c % 2
        sl = slice(g * GC, (g + 1) * GC)
        wq, wk, wv = qkv_w[0:C][sl], qkv_w[C:2 * C][sl], qkv_w[2 * C:3 * C][sl]
        # [o, c] -> tiles [p, ot, cc, oo]: element (cc*128+p, ot*128+oo)
        w_all = np.concatenate([wq, wk, wv], 0)          # [1536, 1024]
        wt = w_all.T.reshape(CK, 128, 12, 128).transpose(1, 2, 0, 3)
        wt = np.ascontiguousarray(wt.reshape(128, 12 * CK * 128))
        in_maps.append(
            {
                "xT": np.ascontiguousarray(x[b].T).astype(bf16),
                "wqkvT": wt.astype(bf16),
                "projT": np.ascontiguousarray(proj_w[:, sl].T).astype(bf16),
                "tril": tril,
                "ident": ident,
                "onesb": onesb,
            }
        )
    return in_maps


def kernel(x, qkv_w, proj_w, proj_b):
    proj_b = np.asarray(proj_b, dtype=np.float32)

    if "nc" not in _cache:
        _cache["nc"] = _build_nc()
    nc = _cache["nc"]

    in_maps = make_in_maps(x, qkv_w, proj_w)
    res = run_bass_kernel_spmd(nc, in_maps, core_ids=list(range(8)))
    out = np.stack(
        [
            (res.results[2 * b]["outT"] + res.results[2 * b + 1]["outT"]).T
            for b in range(B)
        ],
        0,
    )
    return (out + proj_b[None, None, :]).astype(np.float32)


# revision 28
# speedup vs baseline: 1.9637x; 1.9637x over previous
"""Causal attention layer (B=4, N=2048, C=1024, H=16, D=64) on 8 TRN2 NeuronCores.

Sharding: core c -> (batch b = c//2, head-group g = c%2 of 8 heads).

All PE operands are bf16 (halves LDWEIGHTS time vs fp32r; no fp32r narrow-
moving penalty). Every matmul dst is confined to one PSUM bank (512 f32 cols).

  qkv   : per o-tile / 512-col half: accumulate 8 [128,128] w-chunks against
          x chunks -> psum [128,512] -> sbuf (bf16).
  attn  : per (head, 1024-q-megablock): k-outer loop. S_k = kT-tile^T qT in
          1-2 bank-piece matmuls (ap shrinks toward the diagonal), ONE
          full-width exp per k on ScalarE, tril-mask on DVE for diagonal
          tiles, AV accumulates [1|v]^T P into one psum [128,1024] with
          per-piece suffix ranges (causality at 128-key granularity). Each
          512-half is normalized as soon as its last AV lands so proj can
          start early.
  proj  : flipped: stationary = proj chunk, moving = attn_outT -> output is
          TRANSPOSED [C, N]; host transposes back (host time is free).

qkv/transpose/proj work is queued as single-matmul filler closures and
interleaved into the attention k-loop so the PE never waits on ScalarE.
"""
import sys

sys.path.insert(0, "/opt/trn_rl_repo")

import numpy as np

import concourse.bass as bass  # noqa: F401
import concourse.tile as tile
from concourse import bacc, mybir
from concourse.bass_utils import run_bass_kernel_spmd

F32 = mybir.dt.float32
BF16 = mybir.dt.bfloat16
EXP = mybir.ActivationFunctionType.Exp

B, N, C, H, D = 4, 2048, 1024, 16, 64
G = 8            # heads per core
GC = G * D       # 512 channels per core
NT = N // 128    # 16 k-tiles
CK = C // 128    # 8 contraction chunks
MB = 1024        # q-megablock width
SCALE = float(D) ** -0.5

_cache = {}


def _bank_pieces(c0, c1):
    pieces = []
    c = c0
    while c < c1:
        e = min(c1, (c // 512 + 1) * 512)
        pieces.append((c, e))
        c = e
    return pieces


def _build_nc():
    from contextlib import ExitStack

    nc = bacc.Bacc("TRN2", target_bir_lowering=False, debug=False)

    xT_d = nc.dram_tensor("xT", [C, N], BF16, kind="ExternalInput")
    # host pre-tiles qkv weights to [p, ot, cc, oo] so the DMA is contiguous
    wqkvT_d = nc.dram_tensor("wqkvT", [128, 12 * CK * 128], BF16,
                             kind="ExternalInput")
    projT_d = nc.dram_tensor("projT", [GC, C], BF16, kind="ExternalInput")
    tril_d = nc.dram_tensor("tril", [128, 128], BF16, kind="ExternalInput")
    ident_d = nc.dram_tensor("ident", [128, 128], BF16, kind="ExternalInput")
    onesb_d = nc.dram_tensor("onesb", [128, NT], BF16, kind="ExternalInput")
    outT_d = nc.dram_tensor("outT", [C, N], F32, kind="ExternalOutput")

    with tile.TileContext(nc) as tc:
        with ExitStack() as ctx:
            consts = ctx.enter_context(tc.tile_pool(name="consts", bufs=1))
            wt_pool = ctx.enter_context(tc.tile_pool(name="wt", bufs=1))
            xs_pool = ctx.enter_context(tc.tile_pool(name="xs", bufs=1))
            qk_pool = ctx.enter_context(tc.tile_pool(name="qk", bufs=6))
            vT_pool = ctx.enter_context(tc.tile_pool(name="vT", bufs=2))
            vext_pool = ctx.enter_context(tc.tile_pool(name="vext", bufs=1))
            pt_pool = ctx.enter_context(tc.tile_pool(name="pt", bufs=4))
            aoT_pool = ctx.enter_context(tc.tile_pool(name="aoT", bufs=1))
            pj_pool = ctx.enter_context(tc.tile_pool(name="pj", bufs=1))
            os_pool = ctx.enter_context(tc.tile_pool(name="os", bufs=3))
            rf_pool = ctx.enter_context(tc.tile_pool(name="rf", bufs=3))
            bcs_pool = ctx.enter_context(tc.tile_pool(name="bcs", bufs=3))
            tmp_pool = ctx.enter_context(tc.tile_pool(name="tmp", bufs=3))
            ob_pool = ctx.enter_context(tc.tile_pool(name="ob", bufs=3))
            psS = ctx.enter_context(tc.tile_pool(name="psS", bufs=2, space="PSUM"))
            psO = ctx.enter_context(tc.tile_pool(name="psO", bufs=1, space="PSUM"))
            psF = ctx.enter_context(tc.tile_pool(name="psF", bufs=2, space="PSUM"))

            # Input DMA descriptor generation costs ~600ns per dma_start on
            # an engine sequencer. Spread input loads across queues that are
            # idle at the prologue, giving each engine only what must land
            # before that engine's first compute op.
            tril_sb = consts.tile([128, 128], BF16)
            ident_sb = consts.tile([128, 128], BF16)
            wt_all = wt_pool.tile([128, 12, CK * 128], BF16, tag="wt",
                                  name="wt")
            xs = xs_pool.tile([128, CK, N], BF16, tag="xs", name="xs")
            v_ext = [vext_pool.tile([128, NT, 128], BF16, tag=f"ve{h}",
                                    name=f"ve{h}")
                     for h in range(G)]
            pj_sb = pj_pool.tile([128, 4, C], BF16, tag="pj", name="pj")

            def load_w(q, ot):
                q.dma_start(wt_all[:, ot, :],
                            wqkvT_d[:, 1024 * ot:1024 * (ot + 1)])

            def w_stat(ot, cc):
                return wt_all[:, ot, 128 * cc:128 * (cc + 1)]

            def load_x_all(q, c0):
                # one dma_start per 512-col block of all 8 cc chunks:
                # descriptor generation is the bottleneck, not bandwidth
                q.dma_start(
                    xs[:, :, c0:c0 + 512],
                    xT_d[:, c0:c0 + 512].rearrange("(cc p) n -> p cc n",
                                                   p=128),
                )

            # vector queue: consts + v0 weights + late v_ext ones (free
            # until the first qkv-psum copy at ~12us)
            # scalar queue: prologue-critical consts/weights + pair0 ones
            # (only engines SP/Activation/gpsimd may initiate DMAs)
            nc.scalar.dma_start(tril_sb[:], tril_d[:])
            nc.scalar.dma_start(ident_sb[:], ident_d[:])
            load_w(nc.scalar, 8)
            load_w(nc.scalar, 0)
            load_w(nc.scalar, 4)
            for h in range(2):
                nc.scalar.dma_start(v_ext[h][:, :, 0:1], onesb_d[:, :, None])
            # gpsimd queue: x megablock 0 (free until first broadcast ~20us)
            load_x_all(nc.gpsimd, 0)
            load_x_all(nc.gpsimd, 512)
            # sync queue: the rest (x mb1 first: fillers need it ~20us)
            load_x_all(nc.sync, 1024)
            load_x_all(nc.sync, 1536)
            for h in range(2, G):
                nc.sync.dma_start(v_ext[h][:, :, 0:1], onesb_d[:, :, None])
            for ot in (9, 1, 5, 10, 2, 6, 11, 3, 7):
                load_w(nc.sync, ot)
            for gcc in range(4):
                nc.sync.dma_start(
                    pj_sb[:, gcc, :], projT_d[128 * gcc:128 * (gcc + 1), :]
                )

            aoT = [aoT_pool.tile([128, N], BF16, tag=f"ao{p}", name=f"ao{p}")
                   for p in range(4)]

            # ---------------- filler units (1 PE op per closure) -----------
            def qkv_unit(ot, c0, dst):
                """8 closures: accumulate psum [128,512], copy to dst cols."""
                cell = {}
                steps = []
                for cc in range(CK):
                    def _mm(cc=cc, ot=ot, c0=c0, dst=dst):
                        if cc == 0:
                            cell["ps"] = psF.tile([128, 512], F32, tag="F",
                                                  name=f"ps{ot}_{c0}")
                        nc.tensor.matmul(
                            cell["ps"][:],
                            w_stat(ot, cc),
                            xs[:, cc, c0:c0 + 512],
                            start=(cc == 0),
                            stop=(cc == CK - 1),
                        )
                        if cc == CK - 1:
                            nc.vector.tensor_copy(
                                dst[:, c0:c0 + 512], cell["ps"][:]
                            )
                    steps.append(_mm)
                return steps

            def tr_unit(vp, vt, nt):
                """1 closure: transpose one 128-col v tile into v_ext."""
                def _tr(vp=vp, vt=vt, nt=nt):
                    tp = psF.tile([128, 128], BF16, tag="F", name="tp")
                    nc.tensor.transpose(
                        tp[:], vt[:, 128 * nt:128 * (nt + 1)], ident_sb[:]
                    )
                    nc.vector.tensor_copy(
                        v_ext[2 * vp][:, nt, 64:128], tp[:, 0:64]
                    )
                    nc.vector.tensor_copy(
                        v_ext[2 * vp + 1][:, nt, 64:128], tp[:, 64:128]
                    )
                return [_tr]

            def proj_unit(co, c0, pool=None, tag="F"):
                """4 closures: accumulate 4 gc-chunks, copy+DMA out transposed."""
                cell = {}
                steps = []
                for gcc in range(4):
                    def _mm(gcc=gcc, co=co, c0=c0, pool=pool, tag=tag):
                        if gcc == 0:
                            cell["ps"] = (pool or psF).tile(
                                [128, 512], F32, tag=tag, name=f"pp{co}_{c0}"
                            )
                        nc.tensor.matmul(
                            cell["ps"][:],
                            pj_sb[:, gcc, 128 * co:128 * (co + 1)],
                            aoT[gcc][:, c0:c0 + 512],
                            start=(gcc == 0),
                            stop=(gcc == 3),
                        )
                        if gcc == 3:
                            ob = ob_pool.tile([128, 512], F32, tag="ob",
                                              name="ob")
                            nc.vector.tensor_copy(ob[:], cell["ps"][:])
                            # mb1 units run at the tail when ScalarE is idle:
                            # alternate queues there so the final drain isn't
                            # serialized on one descriptor generator
                            dq = nc.scalar if c0 >= MB and co % 2 else nc.sync
                            dq.dma_start(
                                outT_d[128 * co:128 * (co + 1), c0:c0 + 512],
                                ob[:],
                            )
                    steps.append(_mm)
                return steps

            pending = []

            def fill(n):
                for _ in range(min(n, len(pending))):
                    pending.pop(0)()

            # ---------------- attention chain ------------------------------
            def chain(p, h, mb, qT, kT, fps):
                hg = 2 * p + h
                hh = slice(64 * h, 64 * (h + 1))
                kmax = 8 * (mb + 1)
                klo = 8 * mb + 3      # last k writing cols [0,512)
                Ps = [None] * kmax
                c0s = [0] * kmax

                def emit_s_exp(k):
                    c0 = max(0, 128 * k - MB * mb)
                    c0s[k] = c0
                    w = MB - c0
                    S = psS.tile([128, MB], F32, tag="S", name=f"S{k}")
                    for a, b in _bank_pieces(c0, MB):
                        nc.tensor.matmul(
                            S[:, a:b],
                            kT[hh, 128 * k:128 * (k + 1)],
                            qT[hh, MB * mb + a:MB * mb + b],
                        )
                    P = pt_pool.tile([128, MB], BF16, tag="P", name=f"P{k}")
                    nc.scalar.activation(P[:, c0:MB], S[:, c0:MB], EXP,
                                         scale=SCALE)
                    if 128 * k >= MB * mb:
                        nc.vector.tensor_mul(P[:, c0:c0 + 128],
                                             P[:, c0:c0 + 128], tril_sb[:])
                    Ps[k] = P

                def norm_half(oT, hf):
                    oS = os_pool.tile([128, 512], F32, tag="os", name="oS")
                    nc.vector.tensor_copy(oS[:], oT[:, 512 * hf:512 * (hf + 1)])
                    Rf = rf_pool.tile([1, 512], F32, tag="rf", name="Rf")
                    nc.vector.reciprocal_approx_fast(Rf[:], oS[0:1, :])
                    bcs = bcs_pool.tile([128, 512], F32, tag="bcs", name="bcs")
                    nc.gpsimd.partition_broadcast(bcs[:], Rf[:])
                    tmp = tmp_pool.tile([128, 512], BF16, tag="tmp", name="tmp")
                    nc.vector.tensor_mul(tmp[64:128, :], oS[64:128, :],
                                         bcs[64:128, :])
                    nc.sync.dma_start(
                        aoT[p][64 * h:64 * (h + 1),
                               MB * mb + 512 * hf:MB * mb + 512 * (hf + 1)],
                        tmp[64:128, :],
                    )

                oT = psO.tile([128, MB], F32, tag="O", name="oT")
                emit_s_exp(0)
                if kmax > 1:
                    emit_s_exp(1)
                for k in range(kmax):
                    fill(fps)
                    c0 = c0s[k]
                    for a, b in _bank_pieces(c0, MB):
                        stop = (k == klo and a < 512) or \
                               (k == kmax - 1 and b > 512)
                        nc.tensor.matmul(
                            oT[:, a:b],
                            v_ext[hg][:, k, :],
                            Ps[k][:, a:b],
                            start=(k == 0),
                            stop=stop,
                            skip_group_check=True,
                        )
                    if k + 2 < kmax:
                        emit_s_exp(k + 2)
                    if k == klo:
                        norm_half(oT, 0)
                if kmax == 1:
                    norm_half(oT, 0)
                norm_half(oT, 1)

            # ---------------- prologue -------------------------------------
            qTs, kTs, vTs = {}, {}, {}
            for p in range(4):
                qTs[p] = qk_pool.tile([128, N], BF16, tag="qk", name=f"q{p}")
                kTs[p] = qk_pool.tile([128, N], BF16, tag="qk", name=f"k{p}")
            for p in range(4):
                vTs[p] = vT_pool.tile([128, N], BF16, tag="vt", name=f"v{p}")

            def emit(steps):
                for s in steps:
                    s()

            emit(qkv_unit(8, 0, vTs[0]))               # v0 q-half 0
            emit(qkv_unit(8, 512, vTs[0]))             # v0 q-half 1
            emit(qkv_unit(0, 0, qTs[0]))               # q0 half 0
            emit(qkv_unit(0, 512, qTs[0]))             # q0 half 1
            emit(qkv_unit(4, 0, kTs[0]))               # k0 half 0
            emit(qkv_unit(4, 512, kTs[0]))             # k0 half 1
            for nt in range(4):
                emit(tr_unit(0, vTs[0], nt))

            # remainder of pair0's deps (60 steps), consumed early in pair0
            for nt in range(4, 8):
                pending += tr_unit(0, vTs[0], nt)
            for c0 in (1024, 1536):
                pending += qkv_unit(8, c0, vTs[0])
            for c0 in (1024, 1536):
                pending += qkv_unit(0, c0, qTs[0])
            for c0 in (1024, 1536):
                pending += qkv_unit(4, c0, kTs[0])
            for nt in range(8, 16):
                pending += tr_unit(0, vTs[0], nt)
            # pair p block (112 steps each) drains during pair p-1
            for p in range(1, 4):
                for c0 in (0, 512, 1024, 1536):
                    pending += qkv_unit(8 + p, c0, vTs[p])
                for c0 in (0, 512, 1024, 1536):
                    pending += qkv_unit(p, c0, qTs[p])
                for nt in range(NT):
                    pending += tr_unit(p, vTs[p], nt)
                for c0 in (0, 512, 1024, 1536):
                    pending += qkv_unit(4 + p, c0, kTs[p])

            # ---------------- pair loop ------------------------------------
            # fills per attention k-step; pair0 drains 172 queued steps
            # (its 60-step tail + pair1's 112), pairs 1-2 the next 112-block,
            # pair3 the proj-mb0 block (64) during its mb1 chains.
            FPS = {0: (4, 4), 1: (3, 2), 2: (3, 2), 3: (0, 2)}
            for p in range(4):
                for mb in (0, 1):
                    for h in (0, 1):
                        chain(p, h, mb, qTs[p], kTs[p], FPS[p][mb])
                    if p == 3:
                        # mb1 units run at the tail: rotate through the
                        # then-idle psS bufs too, so units double-buffer.
                        for i, (co, hf) in enumerate(
                            (co, hf) for co in range(CK) for hf in (0, 1)
                        ):
                            if mb == 0 or i % 2 == 0:
                                pending += proj_unit(co, MB * mb + 512 * hf)
                            else:
                                pending += proj_unit(co, MB * mb + 512 * hf,
                                                     pool=psS, tag="S")
            fill(len(pending))

    nc.compile()
    return nc


def _tril_np():
    import ml_dtypes

    i = np.arange(128)[:, None]
    j = np.arange(128)[None, :]
    return (j >= i).astype(np.float32).astype(ml_dtypes.bfloat16)


def make_in_maps(x, qkv_w, proj_w):
    import ml_dtypes

    bf16 = ml_dtypes.bfloat16
    x = np.asarray(x, dtype=np.float32)
    qkv_w = np.asarray(qkv_w, dtype=np.float32)
    proj_w = np.asarray(proj_w, dtype=np.float32)
    tril = _tril_np()
    ident = np.eye(128, dtype=np.float32).astype(bf16)
    onesb = np.ones((128, NT), dtype=np.float32).astype(bf16)
    in_maps = []
    for c in range(8):
        b, g = c // 2, c % 2
        sl = slice(g * GC, (g + 1) * GC)
        wq, wk, wv = qkv_w[0:C][sl], qkv_w[C:2 * C][sl], qkv_w[2 * C:3 * C][sl]
        # [o, c] -> tiles [p, ot, cc, oo]: element (cc*128+p, ot*128+oo)
        w_all = np.concatenate([wq, wk, wv], 0)          # [1536, 1024]
        wt = w_all.T.reshape(CK, 128, 12, 128).transpose(1, 2, 0, 3)
        wt = np.ascontiguousarray(wt.reshape(128, 12 * CK * 128))
        in_maps.append(
            {
                "xT": np.ascontiguousarray(x[b].T).astype(bf16),
                "wqkvT": wt.astype(bf16),
                "projT": np.ascontiguousarray(proj_w[:, sl].T).astype(bf16),
                "tril": tril,
                "ident": ident,
                "onesb": onesb,
            }
        )
    return in_maps


def kernel(x, qkv_w, proj_w, proj_b):
    proj_b = np.asarray(proj_b, dtype=np.float32)

    if "nc" not in _cache:
        _cache["nc"] = _build_nc()
    nc = _cache["nc"]

    in_maps = make_in_maps(x, qkv_w, proj_w)
    res = run_bass_kernel_spmd(nc, in_maps, core_ids=list(range(8)))
    out = np.stack(
        [
            (res.results[2 * b]["outT"] + res.results[2 * b + 1]["outT"]).T
            for b in range(B)
        ],
        0,
    )
    return (out + proj_b[None, None, :]).astype(np.float32)
